# revision 2
# baseline (speedup 1.0000x reference)
"""Trainium2 Bass kernel for nn_DimRnn (ragged RNN scan + projections).

Reference computation (B=16, T=512, E=2048, H=1024, D=128):
    xW = x @ W_ih.T + b_ih + b_hh            [B,T,H]
    h chains over ALL batch elements' valid prefixes (lengths[b] tokens
    each):  h = tanh(xW[b,t] + W_hh @ h)
    out[b] = h_after_element_b @ W_l1.T + b_l1   -> [B, D]

Strategy:
  - Host compacts the ragged tokens (only sum(lengths) matter) and
    transposes; 8-core SPMD GEMM computes xw for all valid tokens.
  - The unsplittable scan runs on core 0 in chunks of 256 steps/launch
    (bf16 weights => fast PE weight-load; history-buffer layout makes
    every launch the same NEFF).
  - A tiny head GEMM launch computes the [16,128] output.
All FLOPs (projection GEMM, scan matvecs+tanh, head GEMM, bias adds)
run on Trainium; the host only shards/compacts/relayouts.
"""
import numpy as np
import ml_dtypes
from contextlib import ExitStack

import concourse.bass as bass
from concourse import mybir
from concourse.bass_utils import run_bass_kernel_spmd

F32 = mybir.dt.float32
BF16 = mybir.dt.bfloat16
TANH = mybir.ActivationFunctionType.Tanh

B, T, E, H, D = 16, 512, 2048, 1024, 128
KC = E // 128            # 16 k-chunks
HC = H // 128            # 8 h-chunks
SCAN_STEPS = 256         # steps per scan launch (PE-stream size limit)
NBLK = 512               # tokens per GEMM psum block

# collected per-launch exec times when tracing (read by test.py)
LAST_EXEC_TIMES = []
TRACE = False


# ---------------------------------------------------------------- GEMM
def build_gemm(n_c):
    """Per-core projection: xw = x_cT.T @ W_ih.T + b_ih + b_hh.
    Inputs: x_cT [E, n_c] f32, w_ihT [E, H] f32, biases [2, H] f32
    (b_ih; b_hh), ones [1, NBLK] f32.  Output: xw [H, n_c] f32."""
    assert n_c % NBLK == 0
    nblocks = n_c // NBLK
    nc = bass.Bass("TRN2", target_bir_lowering=False, debug=False,
                   disable_frame_to_traceback=True)
    x_cT = nc.dram_tensor("x_cT", [E, n_c], F32, kind="ExternalInput").ap()
    w_ihT = nc.dram_tensor("w_ihT", [E, H], F32, kind="ExternalInput").ap()
    biases = nc.dram_tensor("biases", [1, 2 * H], F32, kind="ExternalInput").ap()
    ones = nc.dram_tensor("ones", [1, NBLK], F32, kind="ExternalInput").ap()
    xw = nc.dram_tensor("xw", [H, n_c], F32, kind="ExternalOutput").ap()

    with ExitStack() as ctx:
        x_sb = ctx.enter_context(
            nc.sbuf_tensor("x_sb", [128, KC * n_c], F32))
        w_sb = ctx.enter_context(
            nc.sbuf_tensor("w_sb", [128, KC * H], F32))
        b_sb = ctx.enter_context(nc.sbuf_tensor("b_sb", [1, 2 * H], F32))
        ones_sb = ctx.enter_context(nc.sbuf_tensor("ones_sb", [1, NBLK], F32))
        o_sb = ctx.enter_context(
            nc.sbuf_tensor("o_sb", [128, HC * NBLK], F32))
        psb = [ctx.enter_context(nc.psum_tensor(f"ps{i}", [128, NBLK], F32))
               for i in range(8)]
        dma_sem = ctx.enter_context(nc.semaphore("dma_sem"))
        pe_sem = ctx.enter_context(nc.semaphore("pe_sem"))
        act_sem = ctx.enter_context(nc.semaphore("act_sem"))
        block = ctx.enter_context(nc.Block())
        n_in = KC + KC + 1 + 1  # x(16) + w(16) + biases + ones

        @block.sync
        def _(sync):
            for k in range(KC):
                sync.dma_start(
                    out=x_sb[:, k * n_c:(k + 1) * n_c],
                    in_=x_cT[k * 128:(k + 1) * 128, :],
                ).then_inc(dma_sem, 16)
            for k in range(KC):
                sync.dma_start(
                    out=w_sb[:, k * H:(k + 1) * H],
                    in_=w_ihT[k * 128:(k + 1) * 128, :],
                ).then_inc(dma_sem, 16)
            sync.dma_start(out=b_sb[:], in_=biases[:, :]).then_inc(dma_sem, 16)
            sync.dma_start(out=ones_sb[:], in_=ones[:, :]).then_inc(dma_sem, 16)
            for nb in range(nblocks):
                for i in range(HC):
                    sync.wait_ge(act_sem, nb * HC + i + 1)
                    sync.dma_start(
                        out=xw[i * 128:(i + 1) * 128,
                               nb * NBLK:(nb + 1) * NBLK],
                        in_=o_sb[:, i * NBLK:(i + 1) * NBLK],
                    ).then_inc(dma_sem, 16)

        @block.tensor
        def _(tensor):
            tensor.wait_ge(dma_sem, 16 * n_in)
            for nb in range(nblocks):
                for i in range(HC):
                    # bank WAR vs ACT copy of the previous block
                    if nb > 0:
                        tensor.wait_ge(act_sem, (nb - 1) * HC + i + 1)
                    for k in range(KC):
                        nc.tensor.matmul(
                            psb[i][:, :],
                            w_sb[:, k * H + i * 128:k * H + (i + 1) * 128],
                            x_sb[:, k * n_c + nb * NBLK:
                                 k * n_c + (nb + 1) * NBLK],
                            start=(k == 0), stop=False)
                    nc.tensor.matmul(
                        psb[i][:, :],
                        b_sb[0:1, i * 128:(i + 1) * 128],
                        ones_sb[0:1, :],
                        start=False, stop=False)
                    nc.tensor.matmul(
                        psb[i][:, :],
                        b_sb[0:1, H + i * 128:H + (i + 1) * 128],
                        ones_sb[0:1, :],
                        start=False, stop=True).then_inc(pe_sem, 1)

        @block.scalar
        def _(scalar):
            for nb in range(nblocks):
                for i in range(HC):
                    scalar.wait_ge(pe_sem, nb * HC + i + 1)
                    # WAR vs previous block's out-DMA of this o_sb slice
                    if nb > 0:
                        scalar.wait_ge(
                            dma_sem, 16 * (n_in + (nb - 1) * HC + i + 1))
                    nc.scalar.copy(
                        o_sb[:, i * NBLK:(i + 1) * NBLK],
                        psb[i][:, :]).then_inc(act_sem, 1)

    return nc


# ---------------------------------------------------------------- scan
def build_scan_chunk():
    """One scan launch: SCAN_STEPS steps, h_in -> history.
    Inputs: w_hhT [H, H] bf16 (W_hh.T), xw [128, 8*SCAN_STEPS] f32
    (xw[:, t*8+i] = chunk i of token t), h_in [128, 8] bf16.
    Output: hist [128, 8*SCAN_STEPS] bf16 (h after each step)."""
    S = SCAN_STEPS
    nc = bass.Bass("TRN2", target_bir_lowering=False, debug=False,
                   disable_frame_to_traceback=True)
    w_hhT = nc.dram_tensor("w_hhT", [H, H], BF16, kind="ExternalInput").ap()
    xw = nc.dram_tensor("xw", [128, 8 * S], F32, kind="ExternalInput").ap()
    h_in = nc.dram_tensor("h_in", [128, 8], BF16, kind="ExternalInput").ap()
    hist = nc.dram_tensor("hist", [128, 8 * S], BF16,
                          kind="ExternalOutput").ap()

    with ExitStack() as ctx:
        w_sb = ctx.enter_context(nc.sbuf_tensor("w_sb", [128, 8192], BF16))
        xw_sb = ctx.enter_context(nc.sbuf_tensor("xw_sb", [128, 8 * S], F32))
        hi_sb = ctx.enter_context(nc.sbuf_tensor("hi_sb", [128, 8], BF16))
        hist_sb = ctx.enter_context(
            nc.sbuf_tensor("hist_sb", [128, 8 * S], BF16))
        psb = [ctx.enter_context(nc.psum_tensor(f"ps{i}", [128, 1], F32))
               for i in range(8)]
        dma_sem = ctx.enter_context(nc.semaphore("dma_sem"))
        pe_sem = ctx.enter_context(nc.semaphore("pe_sem"))
        act_sem = ctx.enter_context(nc.semaphore("act_sem"))
        block = ctx.enter_context(nc.Block())
        n_in = 8 + 1 + 1

        def h_col(t, j):
            """AP of h chunk j after step t (t=-1 -> h_in)."""
            if t < 0:
                return hi_sb[:, j:j + 1]
            return hist_sb[:, t * 8 + j:t * 8 + j + 1]

        @block.sync
        def _(sync):
            for j in range(8):
                sync.dma_start(
                    out=w_sb[:, j * 1024:(j + 1) * 1024],
                    in_=w_hhT[j * 128:(j + 1) * 128, :],
                ).then_inc(dma_sem, 16)
            sync.dma_start(out=xw_sb[:], in_=xw[:, :]).then_inc(dma_sem, 16)
            sync.dma_start(out=hi_sb[:], in_=h_in[:, :]).then_inc(dma_sem, 16)
            sync.wait_ge(act_sem, 8 * S)
            sync.dma_start(out=hist[:, :], in_=hist_sb[:]).then_inc(dma_sem, 16)

        @block.tensor
        def _(tensor):
            tensor.wait_ge(dma_sem, 16 * n_in)
            for t in range(S):
                base = 8 * (t - 1)
                for m in range(8):
                    if t > 0:
                        tensor.wait_ge(act_sem, base + m + 1)
                    if m < 7:
                        for j in range(m + 1):
                            nc.tensor.matmul(
                                psb[m][:, 0:1],
                                w_sb[:, (j * 8 + m) * 128:
                                     (j * 8 + m + 1) * 128],
                                h_col(t - 1, j),
                                start=(j == 0), stop=False)
                        for i in range(m):
                            nc.tensor.matmul(
                                psb[i][:, 0:1],
                                w_sb[:, (m * 8 + i) * 128:
                                     (m * 8 + i + 1) * 128],
                                h_col(t - 1, m),
                                start=False, stop=False)
                    else:
                        for i in range(7):
                            nc.tensor.matmul(
                                psb[i][:, 0:1],
                                w_sb[:, (7 * 8 + i) * 128:
                                     (7 * 8 + i + 1) * 128],
                                h_col(t - 1, 7),
                                start=False, stop=True).then_inc(pe_sem, 1)
                        for j in range(8):
                            mm = nc.tensor.matmul(
                                psb[7][:, 0:1],
                                w_sb[:, (j * 8 + 7) * 128:
                                     (j * 8 + 7 + 1) * 128],
                                h_col(t - 1, j),
                                start=(j == 0), stop=(j == 7))
                            if j == 7:
                                mm.then_inc(pe_sem, 1)

        @block.scalar
        def _(scalar):
            scalar.wait_ge(dma_sem, 16 * n_in)
            for t in range(S):
                for i in range(8):
                    scalar.wait_ge(pe_sem, 8 * t + i + 1)
                    nc.scalar.activation(
                        hist_sb[:, t * 8 + i:t * 8 + i + 1],
                        psb[i][:, 0:1], TANH,
                        bias=xw_sb[:, t * 8 + i:t * 8 + i + 1],
                    ).then_inc(act_sem, 1)

    return nc


# ---------------------------------------------------------------- head
def build_head(nb):
    """out[b] = hs[:,b] @ W_l1.T + b_l1.
    Inputs: hs [128, 8*nb] f32 (hs[:, i*nb+b] = chunk i of element b's
    final h), w_l1T [H, D] f32, b_l1b [nb, D] f32. Output: out [nb, D]."""
    nc = bass.Bass("TRN2", target_bir_lowering=False, debug=False,
                   disable_frame_to_traceback=True)
    hs = nc.dram_tensor("hs", [128, 8 * nb], F32, kind="ExternalInput").ap()
    w_l1T = nc.dram_tensor("w_l1T", [H, D], F32, kind="ExternalInput").ap()
    b_l1b = nc.dram_tensor("b_l1b", [nb, D], F32, kind="ExternalInput").ap()
    out = nc.dram_tensor("out", [nb, D], F32, kind="ExternalOutput").ap()

    with ExitStack() as ctx:
        hs_sb = ctx.enter_context(nc.sbuf_tensor("hs_sb", [128, 8 * nb], F32))
        wl1_sb = ctx.enter_context(nc.sbuf_tensor("wl1_sb", [128, 8 * D], F32))
        bl1_sb = ctx.enter_context(nc.sbuf_tensor("bl1_sb", [nb, D], F32))
        out_sb = ctx.enter_context(nc.sbuf_tensor("out_sb", [nb, D], F32))
        ps = ctx.enter_context(nc.psum_tensor("ps", [nb, D], F32))
        dma_sem = ctx.enter_context(nc.semaphore("dma_sem"))
        pe_sem = ctx.enter_context(nc.semaphore("pe_sem"))
        out_sem = ctx.enter_context(nc.semaphore("out_sem"))
        block = ctx.enter_context(nc.Block())
        n_in = 1 + 8 + 1

        @block.sync
        def _(sync):
            sync.dma_start(out=hs_sb[:], in_=hs[:, :]).then_inc(dma_sem, 16)
            for i in range(8):
                sync.dma_start(
                    out=wl1_sb[:, i * D:(i + 1) * D],
                    in_=w_l1T[i * 128:(i + 1) * 128, :],
                ).then_inc(dma_sem, 16)
            sync.dma_start(out=bl1_sb[:], in_=b_l1b[:, :]).then_inc(dma_sem, 16)
            sync.wait_ge(out_sem, 1)
            sync.dma_start(out=out[:, :], in_=out_sb[:]).then_inc(dma_sem, 16)

        @block.tensor
        def _(tensor):
            tensor.wait_ge(dma_sem, 16 * n_in)
            for i in range(8):
                mm = nc.tensor.matmul(
                    ps[:, :],
                    hs_sb[:, i * nb:(i + 1) * nb],
                    wl1_sb[:, i * D:(i + 1) * D],
                    start=(i == 0), stop=(i == 7))
                if i == 7:
                    mm.then_inc(pe_sem, 1)

        @block.vector
        def _(vector):
            vector.wait_ge(dma_sem, 16 * n_in)
            vector.wait_ge(pe_sem, 1)
            nc.vector.tensor_add(out_sb[:, :], ps[:, :],
                                 bl1_sb[:, :]).then_inc(out_sem, 1)

    return nc


# ------------------------------------------------------------- runner
def _run(nc, in_maps, core_ids):
    res = run_bass_kernel_spmd(nc, in_maps, core_ids=core_ids, trace=TRACE)
    if TRACE:
        LAST_EXEC_TIMES.append(res.exec_time_ns)
    return res.results


_cache = {}


def _get(name, builder, *args):
    key = (name,) + args
    if key not in _cache:
        _cache[key] = builder(*args)
    return _cache[key]


def kernel(x, lengths, W_ih, W_hh, b_ih, b_hh, W_l1, b_l1):
    global LAST_EXEC_TIMES
    LAST_EXEC_TIMES = []
    x = np.asarray(x, np.float32)
    lengths = np.asarray(lengths, np.int32)
    W_ih = np.asarray(W_ih, np.float32)
    W_hh = np.asarray(W_hh, np.float32)
    b_ih = np.asarray(b_ih, np.float32)
    b_hh = np.asarray(b_hh, np.float32)
    W_l1 = np.asarray(W_l1, np.float32)
    b_l1 = np.asarray(b_l1, np.float32)

    # ---- host: compact ragged tokens ----
    lens = np.clip(lengths, 0, T)
    N = int(lens.sum())
    bounds = np.cumsum(lens) - 1          # global index of element b's
    #                                       last valid token (-1 if empty)
    if N == 0:
        out = np.broadcast_to(b_l1, (B, D)).astype(np.float32).copy()
        return out

    x_valid = np.concatenate([x[b, :lens[b], :] for b in range(B)], axis=0)

    # ---- phase 1: projection GEMM on 8 cores ----
    n_c = max(NBLK, int(np.ceil(N / 8 / NBLK)) * NBLK)
    Npad = 8 * n_c
    x_pad = np.zeros((Npad, E), np.float32)
    x_pad[:N] = x_valid
    w_ihT = np.ascontiguousarray(W_ih.T)           # [E, H]
    biases = np.concatenate([b_ih, b_hh])[None, :]  # [1, 2H]
    ones = np.ones((1, NBLK), np.float32)
    nc_g = _get("gemm", build_gemm, n_c)
    in_maps = []
    for c in range(8):
        x_cT = np.ascontiguousarray(x_pad[c * n_c:(c + 1) * n_c, :].T)
        in_maps.append({"x_cT": x_cT, "w_ihT": w_ihT,
                        "biases": biases, "ones": ones})
    res = _run(nc_g, in_maps, list(range(8)))
    xw_full = np.concatenate([res[c]["xw"] for c in range(8)], axis=1)
    xw_full = xw_full[:, :]  # [H, Npad]

    # ---- phase 2: sequential scan on core 0, SCAN_STEPS per launch ----
    S = SCAN_STEPS
    L = int(np.ceil(N / S))
    xw_scan = np.zeros((H, L * S), np.float32)
    xw_scan[:, :N] = xw_full[:, :N]
    # relayout: [H, L*S] -> per launch [128, 8*S], col t*8+i = chunk i
    w_hhT_bf = np.ascontiguousarray(W_hh.T).astype(ml_dtypes.bfloat16)
    nc_s = _get("scan", build_scan_chunk)
    h_carry = np.zeros((128, 8), ml_dtypes.bfloat16)
    hists = []
    for k in range(L):
        blk = xw_scan[:, k * S:(k + 1) * S]            # [H, S]
        xw_blk = np.ascontiguousarray(
            blk.reshape(8, 128, S).transpose(1, 2, 0).reshape(128, 8 * S))
        r = _run(nc_s, [{"w_hhT": w_hhT_bf, "xw": xw_blk,
                         "h_in": h_carry}], [0])
        hist = r[0]["hist"]                            # [128, 8*S] bf16
        hists.append(hist)
        h_carry = np.ascontiguousarray(hist[:, -8:])

    # ---- phase 3: head GEMM ----
    hs = np.zeros((128, 8 * B), np.float32)
    for b in range(B):
        gi = int(bounds[b])
        if gi < 0:
            continue  # length 0: h=0 snapshot
        k, t = gi // S, gi % S
        col = hists[k][:, t * 8:(t + 1) * 8].astype(np.float32)  # [128, 8]
        for i in range(8):
            hs[:, i * B + b] = col[:, i]
    w_l1T = np.ascontiguousarray(W_l1.T)               # [H, D]
    b_l1b = np.broadcast_to(b_l1, (B, D)).astype(np.float32).copy()
    nc_h = _get("head", build_head, B)
    r = _run(nc_h, [{"hs": hs, "w_l1T": w_l1T, "b_l1b": b_l1b}], [0])
    return np.ascontiguousarray(r[0]["out"].astype(np.float32))
